# revision 1
# baseline (speedup 1.0000x reference)
"""GCN/GCDE message-passing kernel for 8 Trainium2 NeuronCores.

out = softplus(norm * (A @ (norm * x)) @ W + bias),  norm = rsqrt(max(deg,1)) (0 if deg==0)

Strategy (dst-sharded graph parallel, streaming halo):
  - 8-way shard by destination node: each core owns N/8 dst rows and the
    edges pointing at them (host buckets edges; uniform => ~E/8 per core).
  - The host performs the "halo exchange of src features" up front: for
    every edge slot it stages the raw src feature row into a dense,
    slot-ordered array xg (pure integer indexing -- no arithmetic). The
    device then only does large sequential DMA reads; there is no
    on-device gather at all.
  - Identity routing: the host arranges edge slots so that slot
    (tile t, partition p) always feeds dst slot p of its 128-dst chunk.
    Chunks are built from dst nodes sorted by degree so tile counts per
    chunk are tight (~3% padding). On-chip aggregation is then a
    PSUM-accumulated matmul with a constant identity lhsT.
  - 4 chunks ride in one matmul (rhs [128, 4*64]) to amortize PE
    dispatch/weight-load; each 128-dst chunk owns a 64-col stripe of the
    group's PSUM accumulator.
  - Per-edge src-side norm is applied on-device (DVE multiply with a
    broadcast AP over the staged src degrees); dst-side norm, the dense
    W transform (PE), bias and softplus (ACT: exp then ln) run per chunk
    on the [128, 64] aggregates. Output leaves the device transposed +
    degree-sorted; the host undoes both.

Host side does integer/index work only (bucketing, degree counting,
sorting, padding, row duplication); all floating-point math runs on the
NeuronCores.
"""

import sys
from contextlib import ExitStack

sys.path.insert(0, "/opt/trn_rl_repo")

import numpy as np

import concourse.bacc as bacc
import concourse.mybir as mybir
from concourse.masks import make_identity
from concourse.tile import TileContext

F32 = mybir.dt.float32
F16 = mybir.dt.float16

ALU = mybir.AluOpType
ACTF = mybir.ActivationFunctionType


def _r128(v):
    return (v + 127) // 128 * 128


class Geom:
    def __init__(self, n_nodes, n_cores, d=64, cpg=4, slab=8, payload="f16", scale_mode="bcast"):
        assert n_nodes % n_cores == 0
        self.N = n_nodes
        self.D = d
        self.CORES = n_cores
        self.NSH = n_nodes // n_cores
        self.CH = _r128(self.NSH) // 128  # 128-dst chunks per core
        self.SLOTS = self.CH * 128
        self.CPG = cpg  # chunks per matmul group (psum stripe count)
        self.GG = (self.CH + cpg - 1) // cpg  # matmul groups
        self.SLAB = slab  # tiles per DMA slab
        self.payload = payload  # "f32" | "f32r" | "f16"
        self.scale_mode = scale_mode  # "bcast" | "expand" | "swap"


def _rank_within_group(keys):
    order = np.argsort(keys, kind="stable")
    sk = keys[order]
    starts = np.r_[0, np.flatnonzero(sk[1:] != sk[:-1]) + 1]
    grp = np.zeros(len(keys), dtype=np.int64)
    grp[starts] = 1
    grp = np.cumsum(grp) - 1
    ranks_sorted = np.arange(len(keys)) - starts[grp]
    ranks = np.empty(len(keys), dtype=np.int64)
    ranks[order] = ranks_sorted
    return ranks


def make_plan(src, dst, geom):
    """Host-side integer work: bucket edges per core, degree-sort dst nodes,
    build the slot->src mapping and the global tile schedule TG."""
    g = geom
    deg_full = np.bincount(dst, minlength=g.N).astype(np.int64)

    cores = []
    Tc = np.zeros((g.CORES, g.GG), dtype=np.int64)
    for c in range(g.CORES):
        lo = c * g.NSH
        m = (dst >= lo) & (dst < lo + g.NSH)
        es, ed = src[m], dst[m] - lo
        deg = np.bincount(ed, minlength=g.NSH)
        perm = np.argsort(-deg, kind="stable")  # local ids, degree desc
        slot_of = np.empty(g.NSH, dtype=np.int64)
        slot_of[perm] = np.arange(g.NSH)
        degsorted = np.zeros(g.GG * g.CPG * 128, dtype=np.int64)
        degsorted[: g.NSH] = deg[perm]
        # group tile need = max degree within the group's CPG*128 slots
        Tc[c] = degsorted.reshape(g.GG, g.CPG * 128).max(axis=1)
        cores.append(dict(es=es, ed=ed, perm=perm, slot_of=slot_of))

    TG = np.maximum(Tc.max(axis=0), 1)  # global (all cores share the schedule)
    baseG = np.r_[0, np.cumsum(TG)][:-1]
    TOT = int(TG.sum())

    plans = []
    for c in range(g.CORES):
        w = cores[c]
        slots = w["slot_of"][w["ed"]]  # dst slot per edge
        t = _rank_within_group(w["ed"])  # tile index per edge
        gg = slots // (g.CPG * 128)
        j4 = (slots // 128) % g.CPG
        p = slots % 128
        # edge -> (row, col-block) of xg [TOT*128, CPG*64]
        rows = (baseG[gg] + t) * 128 + p
        plans.append(
            dict(rows=rows, j4=j4, es=w["es"], perm=w["perm"],
                 deg_slot=deg_full[c * g.NSH + w["perm"]])
        )
    return dict(TG=TG, baseG=baseG, TOT=TOT, plans=plans, deg_full=deg_full)


def _patch_act_tables():
    import concourse.bacc as _bacc

    if getattr(_bacc, "_gcde_tables_patched", False):
        return
    orig = _bacc.get_activation_tables

    def patched(arch):
        tabs = orig(arch)
        keep = "natural_log_exp_and_others"
        if keep in tabs:
            for k in list(tabs.keys()):
                if k != keep:
                    tabs[k] = set()
        return tabs

    _bacc.get_activation_tables = patched
    _bacc._gcde_tables_patched = True


def build_nc(geom, plan):
    _patch_act_tables()
    g = geom
    TG = plan["TG"]
    TOT = plan["TOT"]
    CW = g.CPG * g.D  # matmul/psum width (4 chunks x 64)
    nc = bacc.Bacc("TRN2", target_bir_lowering=False, debug=False)

    # partition-major layouts: row p holds slot data for all tiles -> every
    # DMA is 128 long contiguous descriptors (full SDMA rate)
    xgdt = F16 if g.payload == "f16" else F32
    xg_d = nc.dram_tensor("xg", [128, TOT * CW], xgdt, kind="ExternalInput")
    degg_d = nc.dram_tensor("degg", [128, TOT * g.CPG], F32, kind="ExternalInput")
    degA_d = nc.dram_tensor("degA", [128, g.CH], F32, kind="ExternalInput")
    w_d = nc.dram_tensor("w", [g.D, g.D], F32, kind="ExternalInput")
    bias_d = nc.dram_tensor("bias", [g.D, 1], F32, kind="ExternalInput")
    outT_d = nc.dram_tensor("outT", [g.D, g.SLOTS], F32, kind="ExternalOutput")

    mmdt = {"f32": F32, "f32r": mybir.dt.float32r, "f16": F16}[g.payload]

    with TileContext(nc) as tc, ExitStack() as _st:
        const = _st.enter_context(tc.tile_pool(name="const", bufs=1))
        xp = _st.enter_context(tc.tile_pool(name="xp", bufs=6))
        dp = _st.enter_context(tc.tile_pool(name="dp", bufs=4))
        sp = _st.enter_context(tc.tile_pool(name="sp", bufs=6))
        ep = _st.enter_context(tc.tile_pool(name="ep", bufs=6))
        psG = _st.enter_context(tc.tile_pool(name="psG", bufs=4, space="PSUM"))
        psT = _st.enter_context(tc.tile_pool(name="psT", bufs=2, space="PSUM"))
        small = _st.enter_context(tc.tile_pool(name="small", bufs=4))

        ident = const.tile([128, 128], F32)
        make_identity(nc, ident)
        if g.payload == "f32":
            ident_mm = ident[:]
        else:
            ident_r = const.tile([128, 128], mmdt, tag="identr")
            nc.vector.tensor_copy(ident_r[:], ident[:])
            ident_mm = ident_r[:]
        w_sb = const.tile([g.D, g.D], F32)
        nc.sync.dma_start(w_sb[:], w_d[:, :])
        bias_sb = const.tile([g.D, 1], F32)
        nc.sync.dma_start(bias_sb[:], bias_d[:, :])

        # dst-side norm per A-slot: rsqrt(max(deg,1)) * (deg > 0)
        degA_sb = const.tile([128, g.CH], F32)
        nc.sync.dma_start(degA_sb[:], degA_d[:, :])
        na1 = const.tile([128, g.CH], F32, tag="na1")
        na2 = const.tile([128, g.CH], F32, tag="na2")
        normA = const.tile([128, g.CH], F32, tag="normA")
        # rsqrt(d) = exp(-0.5*ln(d)) -- keeps every ACT func in one LUT table
        nc.vector.tensor_scalar_max(na1[:], degA_sb[:], 1.0)
        nc.scalar.activation(na2[:], na1[:], ACTF.Ln)
        nc.scalar.activation(na1[:], na2[:], ACTF.Exp, scale=-0.5)
        nc.vector.tensor_scalar(na2[:], degA_sb[:], 0.0, None, ALU.is_gt)
        nc.vector.tensor_mul(normA[:], na1[:], na2[:])

        # src-side norm for every slot, computed once upfront:
        # normg[p, t, j] = (deg>0) * rsqrt(max(deg,1)) for slot (t, p, j)
        degg_sb = dp.tile([128, TOT, g.CPG], F32, tag="degg")
        nc.sync.dma_start(degg_sb[:], degg_d[:, :])
        ng1 = dp.tile([128, TOT, g.CPG], F32, tag="ng1")
        ngdt = F16 if g.payload == "f16" else F32
        normg = dp.tile([128, TOT, g.CPG], ngdt, tag="normg")
        ng2 = dp.tile([128, TOT, g.CPG], F32, tag="ng2")
        nc.vector.tensor_scalar_max(ng1[:], degg_sb[:], 1.0)
        nc.scalar.activation(ng2[:], ng1[:], ACTF.Ln)
        nc.scalar.activation(ng1[:], ng2[:], ACTF.Exp, scale=-0.5)
        nc.vector.tensor_scalar(ng2[:], degg_sb[:], 0.0, None, ALU.is_gt)
        nc.vector.tensor_mul(normg[:], ng1[:], ng2[:])

        for gg in range(g.GG):
            T = int(TG[gg])
            ps = psG.tile([128, CW], F32, tag="ps")
            t0 = 0
            while t0 < T:
                S = min(g.SLAB, T - t0)
                tb = int(plan["baseG"][gg]) + t0
                xt = xp.tile([128, g.SLAB, CW], xgdt, tag="xt")
                nc.sync.dma_start(xt[:, :S, :], xg_d[:, tb * CW : (tb + S) * CW])
                xs = sp.tile([128, g.SLAB, CW], mmdt, tag="xs")
                nbc = normg[:, tb : tb + S, :, None].broadcast_to(
                    [128, S, g.CPG, g.D]
                )
                xtv = xt[:, :S, :].rearrange("p t (j f) -> p t j f", j=g.CPG)
                if g.scale_mode == "expand":
                    en = ep.tile([128, g.SLAB, CW], F16, tag="en")
                    nc.vector.tensor_copy(
                        en[:, :S, :].rearrange("p t (j f) -> p t j f", j=g.CPG), nbc
                    )
                    nc.vector.tensor_tensor(
                        xs[:, :S, :], xt[:, :S, :], en[:, :S, :], ALU.mult
                    )
                elif g.scale_mode == "swap":
                    nc.vector.tensor_tensor(xs[:, :S, :], nbc, xtv, ALU.mult)
                else:
                    nc.vector.tensor_tensor(xs[:, :S, :], xtv, nbc, ALU.mult)
                xs_mm = xs[:]
                for t in range(S):
                    nc.tensor.matmul(
                        ps[:], ident_mm, xs_mm[:, t, :],
                        start=(t0 + t == 0), stop=(t0 + t == T - 1),
                    )
                t0 += S

            # epilogue: per chunk in the group
            for j4 in range(g.CPG):
                j = gg * g.CPG + j4
                if j >= g.CH:
                    break
                vA = small.tile([128, g.D], F32, tag="vA")
                nc.vector.tensor_scalar_mul(
                    vA[:], ps[:, j4 * g.D : (j4 + 1) * g.D], normA[:, j : j + 1]
                )
                pT = psT.tile([64, 128], F32, tag="pT")
                nc.tensor.matmul(pT[:], vA[:], ident[:], is_transpose=True)
                aT = small.tile([g.D, 128], F32, tag="aT")
                nc.scalar.copy(aT[:], pT[:])
                pO = psT.tile([64, 128], F32, tag="pO")
                nc.tensor.matmul(pO[:], w_sb[:], aT[:])
                # softplus(z + bias) = ln(1 + exp(z + bias)); |z| stays small
                ez = small.tile([g.D, 128], F32, tag="ez")
                nc.scalar.activation(ez[:], pO[:], ACTF.Exp, bias=bias_sb[:])
                ob = small.tile([g.D, 128], F32, tag="ob")
                nc.scalar.activation(ob[:], ez[:], ACTF.Ln, bias=1.0)
                nc.sync.dma_start(outT_d[:, j * 128 : (j + 1) * 128], ob[:])

    nc.compile()
    return nc


def _in_maps(x, weight, bias, geom, plan):
    g = geom
    x = np.ascontiguousarray(np.asarray(x, dtype=np.float32))
    deg_full_f = plan["deg_full"].astype(np.float32)
    base = {
        "w": np.ascontiguousarray(np.asarray(weight, dtype=np.float32)),
        "bias": np.ascontiguousarray(np.asarray(bias, dtype=np.float32).reshape(g.D, 1)),
    }
    TOT = plan["TOT"]
    maps = []
    for c in range(g.CORES):
        p = plan["plans"][c]
        xdt = np.float16 if g.payload == "f16" else np.float32
        xg = np.zeros((TOT * 128, g.CPG, g.D), dtype=xdt)
        xg[p["rows"], p["j4"]] = x[p["es"]].astype(xdt)
        degg = np.zeros((TOT * 128, g.CPG), dtype=np.float32)
        degg[p["rows"], p["j4"]] = deg_full_f[p["es"]]
        degA = np.zeros(g.SLOTS, dtype=np.float32)
        degA[: g.NSH] = deg_full_f[c * g.NSH + p["perm"]]
        # to partition-major: [128, TOT*...]
        xg_pm = np.ascontiguousarray(
            xg.reshape(TOT, 128, g.CPG * g.D).transpose(1, 0, 2).reshape(128, -1)
        )
        degg_pm = np.ascontiguousarray(
            degg.reshape(TOT, 128, g.CPG).transpose(1, 0, 2).reshape(128, -1)
        )
        maps.append(
            dict(
                base,
                xg=xg_pm,
                degg=degg_pm,
                degA=np.ascontiguousarray(degA.reshape(g.CH, 128).T),
            )
        )
    return maps


def _unshard(outTs, geom, plan):
    g = geom
    out = np.empty((g.N, g.D), dtype=np.float32)
    for c in range(g.CORES):
        perm = plan["plans"][c]["perm"]
        out[c * g.NSH + perm] = outTs[c][:, : g.NSH].T
    return out


def run_sim(inputs, geom):
    from concourse.bass_interp import MultiCoreSim

    plan = make_plan(np.asarray(inputs["src"]), np.asarray(inputs["dst"]), geom)
    nc = build_nc(geom, plan)
    maps = _in_maps(inputs["x"], inputs["weight"], inputs["bias"], geom, plan)
    sim = MultiCoreSim(nc, num_cores=geom.CORES, trace=False)
    cores = list(sim.cores.values())
    for c, core in enumerate(cores):
        for name, arr in maps[c].items():
            core.tensor(name)[:] = arr
    sim.simulate(check_with_hw=False)
    outTs = [np.array(core.tensor("outT")) for core in cores]
    return _unshard(outTs, geom, plan)


def _install_ntff_hook():
    """The agent image's antenv lacks axon_hooks; recreate the ctypes NTFF
    profile hook (mirrors trn_agent_boot) so trace=True yields exec times."""
    import contextlib
    import ctypes
    import types

    import antenv

    if "antenv.axon_hooks" in sys.modules:
        return
    lib = ctypes.CDLL("/opt/axon/libaxon_pjrt.so")
    if not hasattr(lib, "axon_start_nrt_profile"):
        return
    lib.axon_start_nrt_profile.argtypes = [ctypes.POINTER(ctypes.c_int64), ctypes.c_size_t]
    lib.axon_start_nrt_profile.restype = ctypes.c_int64
    lib.axon_stop_nrt_profile.argtypes = [ctypes.c_char_p]
    lib.axon_stop_nrt_profile.restype = ctypes.c_int64

    @contextlib.contextmanager
    def _hook(output_dir, device_ids):
        import jax

        jax.devices()
        if device_ids:
            ids = (ctypes.c_int64 * len(device_ids))(*device_ids)
            rc = lib.axon_start_nrt_profile(ids, len(device_ids))
        else:
            rc = lib.axon_start_nrt_profile(None, 0)
        if rc != 0:
            raise RuntimeError(f"axon_start_nrt_profile rc={rc}")
        try:
            yield
        finally:
            n = lib.axon_stop_nrt_profile(str(output_dir).encode())
            print(f"ntff profile: {n} file(s) -> {output_dir}", file=sys.stderr)

    mod = types.ModuleType("antenv.axon_hooks")
    mod._hook = _hook
    mod.get_axon_ntff_profile_hook = lambda: _hook
    mod.set_axon_ntff_profile_hook = lambda h: None
    sys.modules["antenv.axon_hooks"] = mod
    antenv.axon_hooks = mod


def run_hw(inputs, geom, trace=False):
    from concourse.bass_utils import run_bass_kernel_spmd

    if trace:
        import concourse.bass_utils as _bu

        _install_ntff_hook()
        _bu.upload_artifacts = lambda d: "local://" + str(d)

    plan = make_plan(np.asarray(inputs["src"]), np.asarray(inputs["dst"]), geom)
    nc = build_nc(geom, plan)
    maps = _in_maps(inputs["x"], inputs["weight"], inputs["bias"], geom, plan)
    import tempfile

    tdir = tempfile.mkdtemp(prefix="gcde_trace_") if trace else None
    res = run_bass_kernel_spmd(
        nc, maps, core_ids=list(range(geom.CORES)), trace=trace, tmpdir=tdir
    )
    if trace:
        print("trace dir:", tdir, file=sys.stderr)
    outTs = [r["outT"] for r in res.results]
    out = _unshard(outTs, geom, plan)
    return out, res


def kernel(**inputs):
    geom = Geom(n_nodes=50000, n_cores=8)
    out, _ = run_hw(inputs, geom)
    return out



# revision 3
# speedup vs baseline: 1.4751x; 1.4751x over previous
"""GCN/GCDE message-passing kernel for 8 Trainium2 NeuronCores (v3).

out = softplus(norm * (A @ (norm * x)) @ W + bias),  norm = rsqrt(max(deg,1)) (0 if deg==0)

Two-launch design (dst-sharded graph parallel, fp8 streaming):

  Launch 1 (tiny, ~node-level): each core takes a 6250-node shard of x
  plus integer degrees and computes y8 = fp8_e4m3(norm * x) on-device
  (ACT ln/exp for rsqrt, one DVE broadcast multiply). ~0.4MB out/core.

  Host (integer/index work only): gathers the fp8 *bytes* of y8 into a
  dense, dst-slot-ordered stream xg8 -- the "halo exchange" staged as
  pure indexing. No host float math: the norm multiply and the fp8
  rounding both happened on-device in launch 1.

  Launch 2 (the stream): 8-way shard by destination node. Identity
  routing: slot (tile t, partition p) feeds dst slot p of its 128-dst
  chunk; chunks built from degree-sorted dst nodes. Aggregation is
  PSUM-accumulated DoubleRow fp8 matmuls (two 128-slot k-tiles per
  instruction, stacked identity lhsT) over [128, 2, 256] slabs; 4
  chunks (CPG=4) ride each 256-wide PSUM stripe group. Epilogue per
  group: dst-norm (DVE), paired-chunk PE transposes into a full
  [128, 256] PSUM tile, one ACT copy/cast, block-diag(W, W) matmuls,
  softplus via exp-then-ln with a stacked bias. Output leaves
  transposed + degree-sorted + pair-stacked; host undoes all three.

The per-edge src-norm multiply (the old DVE bottleneck) is gone: it
was hoisted to node level in launch 1, and the aggregation consumes
pre-normalized fp8 rows straight off the DMA stream.
"""

import sys
from contextlib import ExitStack

sys.path.insert(0, "/opt/trn_rl_repo")

import numpy as np
import ml_dtypes

import concourse.bacc as bacc
import concourse.mybir as mybir
from concourse.masks import make_identity
from concourse.tile import TileContext

F32 = mybir.dt.float32
F16 = mybir.dt.float16
F8 = mybir.dt.float8e4

ALU = mybir.AluOpType
ACTF = mybir.ActivationFunctionType
NPF8 = ml_dtypes.float8_e4m3


def _r128(v):
    return (v + 127) // 128 * 128


class Geom:
    def __init__(self, n_nodes=50000, n_cores=8, d=64, cpg=4, slab=8):
        assert n_nodes % n_cores == 0
        self.N = n_nodes
        self.D = d
        self.CORES = n_cores
        self.NSH = n_nodes // n_cores
        self.CH = _r128(self.NSH) // 128     # 128-dst chunks per core (49)
        self.SLOTS = self.CH * 128
        self.CPG = cpg                        # chunks per psum stripe group
        self.GG = (self.CH + cpg - 1) // cpg  # matmul groups (13)
        self.SLAB = slab                      # tiles per DMA slab (even)
        self.CW = cpg * d                     # stream row width (256)
        self.NT = self.SLOTS // 128           # node tiles per shard for launch 1 (49)


def _rank_within_group(keys):
    order = np.argsort(keys, kind="stable")
    sk = keys[order]
    starts = np.r_[0, np.flatnonzero(sk[1:] != sk[:-1]) + 1]
    grp = np.zeros(len(keys), dtype=np.int64)
    grp[starts] = 1
    grp = np.cumsum(grp) - 1
    ranks_sorted = np.arange(len(keys)) - starts[grp]
    ranks = np.empty(len(keys), dtype=np.int64)
    ranks[order] = ranks_sorted
    return ranks


def make_plan(src, dst, geom):
    """Host-side integer work: bucket edges per core, degree-sort dst nodes,
    build the slot->src mapping and the global (even) tile schedule TG."""
    g = geom
    deg_full = np.bincount(dst, minlength=g.N).astype(np.int64)

    cores = []
    Tc = np.zeros((g.CORES, g.GG), dtype=np.int64)
    for c in range(g.CORES):
        lo = c * g.NSH
        m = (dst >= lo) & (dst < lo + g.NSH)
        es, ed = src[m], dst[m] - lo
        deg = np.bincount(ed, minlength=g.NSH)
        perm = np.argsort(-deg, kind="stable")  # local ids, degree desc
        slot_of = np.empty(g.NSH, dtype=np.int64)
        slot_of[perm] = np.arange(g.NSH)
        degsorted = np.zeros(g.GG * g.CPG * 128, dtype=np.int64)
        degsorted[: g.NSH] = deg[perm]
        Tc[c] = degsorted.reshape(g.GG, g.CPG * 128).max(axis=1)
        cores.append(dict(es=es, ed=ed, perm=perm, slot_of=slot_of))

    TG = np.maximum(Tc.max(axis=0), 2)
    TG += TG % 2  # even tile counts: DoubleRow pairs only
    baseG = np.r_[0, np.cumsum(TG)][:-1]
    TOT = int(TG.sum())

    plans = []
    for c in range(g.CORES):
        w = cores[c]
        slots = w["slot_of"][w["ed"]]
        t = _rank_within_group(w["ed"])
        gg = slots // (g.CPG * 128)
        j4 = (slots // 128) % g.CPG
        p = slots % 128
        rows = (baseG[gg] + t) * 128 + p
        plans.append(dict(rows=rows, j4=j4, es=w["es"], perm=w["perm"]))
    return dict(TG=TG, baseG=baseG, TOT=TOT, plans=plans, deg_full=deg_full)


def _patch_act_tables():
    import concourse.bacc as _bacc

    if getattr(_bacc, "_gcde_tables_patched", False):
        return
    orig = _bacc.get_activation_tables

    def patched(arch):
        tabs = orig(arch)
        keep = "natural_log_exp_and_others"
        if keep in tabs:
            for k in list(tabs.keys()):
                if k != keep:
                    tabs[k] = set()
        return tabs

    _bacc.get_activation_tables = patched
    _bacc._gcde_tables_patched = True


def _emit_norm(nc, pool, deg_sb, shape, tag):
    """rsqrt(max(deg,1)) * (deg > 0), via exp(-0.5*ln(d)) (one ACT table)."""
    n1 = pool.tile(shape, F32, tag=tag + "1")
    n2 = pool.tile(shape, F32, tag=tag + "2")
    out = pool.tile(shape, F32, tag=tag)
    nc.vector.tensor_scalar_max(n1[:], deg_sb[:], 1.0)
    nc.scalar.activation(n2[:], n1[:], ACTF.Ln)
    nc.scalar.activation(n1[:], n2[:], ACTF.Exp, scale=-0.5)
    nc.vector.tensor_scalar(n2[:], deg_sb[:], 0.0, None, ALU.is_gt)
    nc.vector.tensor_mul(out[:], n1[:], n2[:])
    return out


def build_nc1(geom):
    """Launch 1: y8 = fp8(norm * x) for one 6272-row node shard."""
    _patch_act_tables()
    g = geom
    nc = bacc.Bacc("TRN2", target_bir_lowering=False, debug=False)
    xsh_d = nc.dram_tensor("xsh", [128, g.NT * g.D], F32, kind="ExternalInput")
    degsh_d = nc.dram_tensor("degsh", [128, g.NT], F32, kind="ExternalInput")
    y8_d = nc.dram_tensor("y8", [128, g.NT * g.D], F8, kind="ExternalOutput")

    with TileContext(nc) as tc, ExitStack() as st:
        pool = st.enter_context(tc.tile_pool(name="pool", bufs=1))
        xsh = pool.tile([128, g.NT, g.D], F32, tag="xsh")
        degsh = pool.tile([128, g.NT], F32, tag="degsh")
        nc.sync.dma_start(xsh[:].rearrange("p t f -> p (t f)"), xsh_d[:, :])
        nc.sync.dma_start(degsh[:], degsh_d[:, :])
        normsh = _emit_norm(nc, pool, degsh, [128, g.NT], "n")
        y8 = pool.tile([128, g.NT, g.D], F8, tag="y8")
        nbc = normsh[:, :, None].broadcast_to([128, g.NT, g.D])
        nc.vector.tensor_tensor(y8[:], xsh[:], nbc, ALU.mult)
        nc.sync.dma_start(y8_d[:, :], y8[:].rearrange("p t f -> p (t f)"))
    nc.compile()
    return nc


def build_nc2(geom, plan):
    """Launch 2: fp8 stream -> DoubleRow identity aggregation -> epilogue."""
    _patch_act_tables()
    g = geom
    TG = plan["TG"]
    TOT = plan["TOT"]
    CW = g.CW
    nc = bacc.Bacc("TRN2", target_bir_lowering=False, debug=False)

    xg_d = nc.dram_tensor("xg", [128, TOT * CW], F8, kind="ExternalInput")
    degA_d = nc.dram_tensor("degA", [128, g.GG * g.CPG], F32, kind="ExternalInput")
    w2_d = nc.dram_tensor("w2", [128, 128], F32, kind="ExternalInput")
    bias2_d = nc.dram_tensor("bias2", [128, 1], F32, kind="ExternalInput")
    outT_d = nc.dram_tensor("outT", [128, g.GG * CW], F32, kind="ExternalOutput")

    with TileContext(nc) as tc, ExitStack() as st:
        const = st.enter_context(tc.tile_pool(name="const", bufs=1))
        xp = st.enter_context(tc.tile_pool(name="xp", bufs=6))
        sp = st.enter_context(tc.tile_pool(name="sp", bufs=4))
        psG = st.enter_context(tc.tile_pool(name="psG", bufs=4, space="PSUM"))
        psT = st.enter_context(tc.tile_pool(name="psT", bufs=2, space="PSUM"))
        psO = st.enter_context(tc.tile_pool(name="psO", bufs=2, space="PSUM"))

        ident = const.tile([128, 128], F32)
        make_identity(nc, ident)
        # DoubleRow stationary: two stacked fp8 identities [128, 2, 128]
        id_dr = const.tile([128, 2, 128], F8, tag="id_dr")
        nc.vector.tensor_copy(id_dr[:, 0, :], ident[:])
        nc.vector.tensor_copy(id_dr[:, 1, :], ident[:])

        w2_sb = const.tile([128, 128], F32, tag="w2f32")
        nc.sync.dma_start(w2_sb[:], w2_d[:, :])
        w2h = const.tile([128, 128], F16, tag="w2h")
        nc.vector.tensor_copy(w2h[:], w2_sb[:])
        bias2 = const.tile([128, 1], F32, tag="bias2")
        nc.sync.dma_start(bias2[:], bias2_d[:, :])

        degA_sb = const.tile([128, g.GG * g.CPG], F32, tag="degA")
        nc.sync.dma_start(degA_sb[:], degA_d[:, :])
        normA = _emit_norm(nc, const, degA_sb, [128, g.GG * g.CPG], "na")

        for gg in range(g.GG):
            T = int(TG[gg])
            ps = psG.tile([128, CW], F32, tag="ps")
            t0 = 0
            while t0 < T:
                S = min(g.SLAB, T - t0)
                tb = int(plan["baseG"][gg]) + t0
                xt = xp.tile([128, g.SLAB, CW], F8, tag="xt")
                nc.sync.dma_start(xt[:, :S, :], xg_d[:, tb * CW : (tb + S) * CW])
                for t in range(0, S, 2):
                    nc.tensor.matmul(
                        ps[:], id_dr[:], xt[:, t : t + 2, :],
                        start=(t0 + t == 0), stop=(t0 + t + 2 == T),
                        perf_mode=mybir.MatmulPerfMode.DoubleRow,
                    )
                t0 += S

            # epilogue for the group's 4 chunks (2 stacked pairs)
            vA = sp.tile([128, g.CPG, g.D], F32, tag="vA")
            nabc = normA[:, gg * g.CPG : (gg + 1) * g.CPG, None].broadcast_to(
                [128, g.CPG, g.D]
            )
            nc.vector.tensor_tensor(
                vA[:], ps[:].rearrange("p (j f) -> p j f", j=g.CPG), nabc, ALU.mult
            )
            pT4 = psT.tile([128, CW], F32, tag="pT4")
            for pr in range(2):
                # transpose a chunk PAIR [128 slots, 2*64 feats] at once:
                # output partitions 0-63 = chunk 2pr feats, 64-127 = chunk 2pr+1
                nc.tensor.matmul(
                    pT4[:, pr * 128 : (pr + 1) * 128],
                    vA[:, 2 * pr : 2 * pr + 2, :].rearrange("p j f -> p (j f)"),
                    ident[:], is_transpose=True,
                )
            aT4 = sp.tile([128, CW], F16, tag="aT4")
            nc.scalar.copy(aT4[:], pT4[:])
            pO4 = psO.tile([128, CW], F32, tag="pO4")
            for pr in range(2):
                nc.tensor.matmul(
                    pO4[:, pr * 128 : (pr + 1) * 128], w2h[:],
                    aT4[:, pr * 128 : (pr + 1) * 128],
                )
            # softplus(z + bias) = ln(1 + exp(z + bias))
            ez = sp.tile([128, CW], F32, tag="ez")
            nc.scalar.activation(ez[:], pO4[:], ACTF.Exp, bias=bias2[:])
            ob = sp.tile([128, CW], F32, tag="ob")
            nc.scalar.activation(ob[:], ez[:], ACTF.Ln, bias=1.0)
            nc.sync.dma_start(outT_d[:, gg * CW : (gg + 1) * CW], ob[:])

    nc.compile()
    return nc


def _shard_maps_l1(x, deg_full, geom):
    """Per-core launch-1 inputs: partition-major x shard + degrees."""
    g = geom
    x = np.ascontiguousarray(np.asarray(x, dtype=np.float32))
    maps = []
    for c in range(g.CORES):
        lo = c * g.NSH
        xs = np.zeros((g.SLOTS, g.D), dtype=np.float32)
        xs[: g.NSH] = x[lo : lo + g.NSH]
        ds = np.zeros(g.SLOTS, dtype=np.float32)
        ds[: g.NSH] = deg_full[lo : lo + g.NSH]
        # node local id = t*128 + p  ->  [p, t, f] partition-major
        xs_pm = np.ascontiguousarray(
            xs.reshape(g.NT, 128, g.D).transpose(1, 0, 2).reshape(128, -1)
        )
        ds_pm = np.ascontiguousarray(ds.reshape(g.NT, 128).T)
        maps.append(dict(xsh=xs_pm, degsh=ds_pm))
    return maps


def _assemble_y8(y8_outs, geom):
    """Reassemble full [N, D] fp8 byte array from launch-1 shard outputs."""
    g = geom
    y8u = np.empty((g.N, g.D), dtype=np.uint8)
    for c in range(g.CORES):
        o = np.asarray(y8_outs[c]).reshape(128, g.NT, g.D)
        ou = o.view(np.uint8) if o.dtype != np.uint8 else o
        # [p, t, f] -> node t*128+p
        full = ou.transpose(1, 0, 2).reshape(g.SLOTS, g.D)
        y8u[c * g.NSH : (c + 1) * g.NSH] = full[: g.NSH]
    return y8u


def _shard_maps_l2(y8u, weight, bias, geom, plan):
    """Per-core launch-2 inputs: fp8 slot stream (pure byte gather) + consts."""
    g = geom
    TOT = plan["TOT"]
    deg_full_f = plan["deg_full"].astype(np.float32)
    w = np.asarray(weight, dtype=np.float32)
    b = np.asarray(bias, dtype=np.float32)
    w2 = np.zeros((128, 128), dtype=np.float32)
    w2[:64, :64] = w
    w2[64:, 64:] = w
    bias2 = np.concatenate([b, b]).reshape(128, 1).astype(np.float32)

    maps = []
    for c in range(g.CORES):
        p = plan["plans"][c]
        xg = np.zeros((TOT * 128, g.CPG, g.D), dtype=np.uint8)
        xg[p["rows"], p["j4"]] = y8u[p["es"]]
        xg_pm = np.ascontiguousarray(
            xg.reshape(TOT, 128, g.CW).transpose(1, 0, 2).reshape(128, -1)
        ).view(NPF8)
        degA = np.zeros(g.GG * g.CPG * 128, dtype=np.float32)
        degA[: g.NSH] = deg_full_f[c * g.NSH + p["perm"]]
        maps.append(
            dict(
                xg=xg_pm,
                degA=np.ascontiguousarray(degA.reshape(g.GG * g.CPG, 128).T),
                w2=w2,
                bias2=bias2,
            )
        )
    return maps


def _unshard(outTs, geom, plan):
    """outT [128, GG*256]: group gg cols [gg*256:(gg+1)*256], chunk c=4*gg+jj at
    cols (jj//2)*128 + slot, partitions 64*(jj%2) + feature."""
    g = geom
    out = np.empty((g.N, g.D), dtype=np.float32)
    for c in range(g.CORES):
        perm = plan["plans"][c]["perm"]
        oT = np.asarray(outTs[c]).reshape(128, g.GG, 2, 128)  # [p, gg, pair, slot]
        # chunk index ch = gg*4 + pair*2 + (p>=64); feature = p%64
        chunks = oT.reshape(2, 64, g.GG, 2, 128)  # [phalf, feat, gg, pair, slot]
        # slot id = ch*128 + slot
        vals = chunks.transpose(2, 3, 0, 4, 1).reshape(g.GG * 4 * 128, g.D)
        out[c * g.NSH + perm] = vals[: g.NSH]
    return out


def _install_ntff_hook():
    """Recreate the ctypes NTFF profile hook (agent image lacks axon_hooks)."""
    import contextlib
    import ctypes
    import types

    import antenv

    if "antenv.axon_hooks" in sys.modules:
        return
    lib = ctypes.CDLL("/opt/axon/libaxon_pjrt.so")
    if not hasattr(lib, "axon_start_nrt_profile"):
        return
    lib.axon_start_nrt_profile.argtypes = [ctypes.POINTER(ctypes.c_int64), ctypes.c_size_t]
    lib.axon_start_nrt_profile.restype = ctypes.c_int64
    lib.axon_stop_nrt_profile.argtypes = [ctypes.c_char_p]
    lib.axon_stop_nrt_profile.restype = ctypes.c_int64

    @contextlib.contextmanager
    def _hook(output_dir, device_ids):
        import jax

        jax.devices()
        if device_ids:
            ids = (ctypes.c_int64 * len(device_ids))(*device_ids)
            rc = lib.axon_start_nrt_profile(ids, len(device_ids))
        else:
            rc = lib.axon_start_nrt_profile(None, 0)
        if rc != 0:
            raise RuntimeError(f"axon_start_nrt_profile rc={rc}")
        try:
            yield
        finally:
            n = lib.axon_stop_nrt_profile(str(output_dir).encode())
            print(f"ntff profile: {n} file(s) -> {output_dir}", file=sys.stderr)

    mod = types.ModuleType("antenv.axon_hooks")
    mod._hook = _hook
    mod.get_axon_ntff_profile_hook = lambda: _hook
    mod.set_axon_ntff_profile_hook = lambda h: None
    sys.modules["antenv.axon_hooks"] = mod
    antenv.axon_hooks = mod


def run_hw(inputs, geom, trace=False):
    from concourse.bass_utils import run_bass_kernel_spmd

    if trace:
        import concourse.bass_utils as _bu

        _install_ntff_hook()
        _bu.upload_artifacts = lambda d: "local://" + str(d)

    g = geom
    src = np.asarray(inputs["src"])
    dst = np.asarray(inputs["dst"])
    plan = make_plan(src, dst, g)

    import tempfile

    # ---- launch 1: y8 = fp8(norm * x) per node shard
    nc1 = build_nc1(g)
    maps1 = _shard_maps_l1(inputs["x"], plan["deg_full"], g)
    tdir1 = tempfile.mkdtemp(prefix="gcde1_") if trace else None
    res1 = run_bass_kernel_spmd(
        nc1, maps1, core_ids=list(range(g.CORES)), trace=trace, tmpdir=tdir1
    )
    y8u = _assemble_y8([r["y8"] for r in res1.results], g)

    # ---- host: fp8 byte gather into the dst-slot stream
    maps2 = _shard_maps_l2(y8u, inputs["weight"], inputs["bias"], g, plan)

    # ---- launch 2: aggregation + epilogue
    nc2 = build_nc2(g, plan)
    tdir2 = tempfile.mkdtemp(prefix="gcde2_") if trace else None
    res2 = run_bass_kernel_spmd(
        nc2, maps2, core_ids=list(range(g.CORES)), trace=trace, tmpdir=tdir2
    )
    if trace:
        print("trace dirs:", tdir1, tdir2, file=sys.stderr)
    out = _unshard([r["outT"] for r in res2.results], g, plan)
    return out, (res1, res2)


def kernel(**inputs):
    geom = Geom()
    out, _ = run_hw(inputs, geom)
    return out
